# revision 9
# baseline (speedup 1.0000x reference)
"""2-layer GCN (PyG GCNConv style) on 8 Trainium2 NeuronCores.

Strategy (graph/node parallel, per sharding hint):
  - Nodes are range-sharded across 8 cores (R = N/8 rows each).
  - All device compute runs in bf16 (fp32 PSUM accumulation); tolerance
    is 2e-2 so bf16 is plenty: 4x PE rate + FWL weight loads + half the
    DMA/collective bytes.
  - Normalization factoring: with dis = deg^-1/2,
        out = Dis . A01^T . Dis . (x @ W) + b
    so the per-edge norm never appears on device:
      * dis[src] is folded into the gathered rows (phase-1/3 PSUM->SBUF
        copy is an activation with per-partition scale=dis).
      * dis[dst] is applied on the aggregated output tile (activation
        scale), with the bias pre-divided via a rank-1 matmul
        lhsT=invdis so that relu((agg + b/dis)*dis) == relu(agg*dis+b).
      * The selection matrix S becomes a pure 0/1 one-hot built with a
        single batched DVE is_equal per destination tile.
  - Per dst-tile (128 nodes), edges are packed into chunks of 128 and
    gathered with TWO dma_gather calls per tile (sources < 32768 from
    the table base, sources >= 32768 from a shifted base, because
    dma_gather indices are int16).  One dma_gather amortizes the ~1us
    SWDGE fixed cost over a whole tile's rows.  Slot i of the index
    list lands at out[i % 128, i // 128, :]; pad slots use index 0 and
    dstl=255 so S zeroes their contribution.
  - Device per core:
      phase 1: xw1' = dis_c * (x_c @ W1)            -> AllGather (bf16)
      phase 2: aggregate -> relu((agg + b1/dis)*dis) -> h1, PE-transpose
               to h1T in SBUF
      phase 3: hw2' = dis_c * (h1 @ W2)             -> AllGather (bf16)
      phase 4: aggregate -> (agg + b2/dis)*dis      -> out (bf16)
  - Host concatenates the 8 row-shards and casts to f32.
"""

import sys

for p in ("/opt/trn_rl_repo",):
    if p not in sys.path:
        sys.path.insert(0, p)

import numpy as np
import ml_dtypes

import concourse.bass as bass
import concourse.bacc as bacc
import concourse.mybir as mybir
import concourse.tile as tile
from concourse import bass_utils
from concourse.masks import make_identity

P = 128
NCORES = 8
BF16 = ml_dtypes.bfloat16
SPLIT = 32768  # int16 index limit for dma_gather


# ----------------------------------------------------------------------------
# Host-side preprocessing
# ----------------------------------------------------------------------------

def _preprocess(x, edge_index, n_cores):
    """Pack per-core edge data for the LO/HI dma_gather scheme.

    Returns per-core tuples (idx16, dstl, dis_tiles, invdis_row) plus the
    shared chunk structure (clo, chi) per tile.
    """
    N = x.shape[0]
    R = N // n_cores
    assert R * n_cores == N
    ntiles = (R + P - 1) // P
    RP = ntiles * P

    src = edge_index[0].astype(np.int64)
    dst = edge_index[1].astype(np.int64)
    loops = np.arange(N, dtype=np.int64)
    src = np.concatenate([src, loops])
    dst = np.concatenate([dst, loops])

    deg = np.bincount(dst, minlength=N).astype(np.float32)
    dis = np.where(deg > 0, 1.0 / np.sqrt(deg), 0.0).astype(np.float32)
    invdis = np.where(deg > 0, np.sqrt(deg), 0.0).astype(np.float32)

    core_id = dst // R
    dloc = dst - core_id * R
    tl = dloc // P
    dstl = (dloc - tl * P).astype(np.float32)
    is_hi = (src >= SPLIT).astype(np.int64)

    # per (core, tile): lo and hi edge counts -> shared chunk counts
    key = (core_id * ntiles + tl) * 2 + is_hi
    counts = np.bincount(key, minlength=n_cores * ntiles * 2)
    counts = counts.reshape(n_cores, ntiles, 2)
    clo = np.ceil(counts[:, :, 0].max(axis=0) / P).astype(np.int64)
    chi = np.ceil(counts[:, :, 1].max(axis=0) / P).astype(np.int64)
    chunks = clo + chi                     # chunks per tile (shared)
    total = int(chunks.sum())
    offs = np.concatenate([[0], np.cumsum(chunks)])      # chunk offsets
    lo_slots = int(clo.sum()) * P          # total LO slot count
    hi_slots = int(chi.sum()) * P
    lo_offs = np.concatenate([[0], np.cumsum(clo)])      # in chunks
    hi_offs = np.concatenate([[0], np.cumsum(chi)])

    packed = []
    for c in range(n_cores):
        m = core_id == c
        s_c = src[m]
        t_c = tl[m]
        d_c = dstl[m]
        h_c = is_hi[m]
        # order edges by (tile, hi, arbitrary)
        order = np.lexsort((h_c, t_c))
        s_c, t_c, d_c, h_c = s_c[order], t_c[order], d_c[order], h_c[order]

        # slot position within the tile's lo/hi group
        grp = t_c * 2 + h_c
        cnt = np.bincount(grp, minlength=ntiles * 2)
        starts = np.cumsum(cnt) - cnt
        pos = np.arange(len(s_c)) - np.repeat(starts, cnt)

        # dstl array: [P, total] with per-tile chunk order [LO..., HI...]
        A_dl = np.full(total * P, 255.0, np.float32)
        chunk_in_tile = np.where(h_c == 0, pos // P, clo[t_c] + pos // P)
        slots = (offs[t_c] + chunk_in_tile) * P + pos % P
        A_dl[slots] = d_c

        # index lists (flat slot order: position i -> chunk i//128, lane i%128)
        idx_lo = np.zeros(lo_slots, np.int16)
        idx_hi = np.zeros(hi_slots, np.int16)
        mlo = h_c == 0
        idx_lo[(lo_offs[t_c[mlo]] * P + pos[mlo]).astype(np.int64)] = \
            s_c[mlo].astype(np.int16)
        mhi = h_c == 1
        idx_hi[(hi_offs[t_c[mhi]] * P + pos[mhi]).astype(np.int64)] = \
            (s_c[mhi] - SPLIT).astype(np.int16)

        def lay_dl(a):
            return np.ascontiguousarray(a.reshape(total, P).T).astype(BF16)

        def lay_idx(a):
            # [16, n/16] wrapped (list pos j -> [j%16, j//16]), replicated
            # across the 128 partitions (each Q7 core reads its own window)
            w = np.ascontiguousarray(a.reshape(-1, 16).T)  # [16, n/16]
            return np.ascontiguousarray(np.tile(w, (8, 1)))

        dis_c = np.zeros(RP, np.float32)
        dis_c[:R] = dis[c * R:(c + 1) * R]
        invdis_c = np.zeros(RP, np.float32)
        invdis_c[:R] = invdis[c * R:(c + 1) * R]
        dis_tiles = np.ascontiguousarray(dis_c.reshape(ntiles, P).T)
        invdis_row = invdis_c.reshape(1, RP).astype(BF16)

        packed.append((lay_idx(idx_lo), lay_idx(idx_hi), lay_dl(A_dl),
                       dis_tiles, invdis_row))
    return (packed, [int(v) for v in clo], [int(v) for v in chi], R, ntiles)


# ----------------------------------------------------------------------------
# Device kernel builder
# ----------------------------------------------------------------------------

def build_nc(N, R, ntiles, clo, chi, F0, F1, F2, n_cores):
    """Build the SPMD Bass program. All dims: F0,F1,F2 multiples of 128."""
    f32 = mybir.dt.float32
    bf16 = mybir.dt.bfloat16
    i16 = mybir.dt.int16
    i32 = mybir.dt.int32
    K0 = F0 // P       # k-tiles in layer-1 matmul
    H1 = F1 // P       # 128-wide halves of F1
    K2 = F1 // P       # k-tiles in layer-2 matmul (= H1)
    assert F2 <= 512 and F2 % P == 0
    last_rows = R - (ntiles - 1) * P
    RP = ntiles * P
    chunks = [a + b for a, b in zip(clo, chi)]
    total = int(sum(chunks))
    maxch = max(chunks)
    offs = [0]
    for c in chunks:
        offs.append(offs[-1] + c)
    lo_offs = [0]
    for c in clo:
        lo_offs.append(lo_offs[-1] + c)
    hi_offs = [0]
    for c in chi:
        hi_offs.append(hi_offs[-1] + c)
    lo_cols = lo_offs[-1] * 8   # int16 idx columns ([16-row blocks] * 8 per chunk)
    hi_cols = hi_offs[-1] * 8

    nc = bacc.Bacc("TRN2", target_bir_lowering=False, debug=False,
                   num_devices=n_cores)

    xT = nc.dram_tensor("xT", [F0, R], bf16, kind="ExternalInput").ap()
    idxlo_d = nc.dram_tensor("idxlo", [P, lo_cols], i16,
                             kind="ExternalInput").ap()
    idxhi_d = nc.dram_tensor("idxhi", [P, hi_cols], i16,
                             kind="ExternalInput").ap()
    dstl_d = nc.dram_tensor("dstl", [P, total], bf16,
                            kind="ExternalInput").ap()
    dis_d = nc.dram_tensor("dis", [P, ntiles], f32, kind="ExternalInput").ap()
    invdis_d = nc.dram_tensor("invdis", [1, RP], bf16,
                              kind="ExternalInput").ap()
    W1_d = nc.dram_tensor("W1", [F0, F1], bf16, kind="ExternalInput").ap()
    b1_d = nc.dram_tensor("b1", [F1], bf16, kind="ExternalInput").ap()
    W2_d = nc.dram_tensor("W2", [F1, F2], bf16, kind="ExternalInput").ap()
    b2_d = nc.dram_tensor("b2", [F2], bf16, kind="ExternalInput").ap()
    out_d = nc.dram_tensor("out", [R, F2], bf16, kind="ExternalOutput").ap()

    rg = [list(range(n_cores))]

    with tile.TileContext(nc) as tc:
        with (
            tc.tile_pool(name="dram", bufs=1, space="DRAM") as dram,
            tc.tile_pool(name="const", bufs=1) as const,
        ):
            ag1_in = dram.tile([R, F1], bf16)
            ag1_out = dram.tile([N, F1], bf16, addr_space="Shared")
            ag2_in = dram.tile([R, F2], bf16)
            ag2_out = dram.tile([N, F2], bf16, addr_space="Shared")

            w1_sb = const.tile([P, K0 * F1], bf16)
            nc.sync.dma_start(
                out=w1_sb[:].rearrange("p (k f) -> p k f", k=K0),
                in_=W1_d.rearrange("(k p) f -> p k f", p=P))
            w2_sb = const.tile([P, K2 * F2], bf16)
            nc.sync.dma_start(
                out=w2_sb[:].rearrange("p (k f) -> p k f", k=K2),
                in_=W2_d.rearrange("(k p) f -> p k f", p=P))
            b1_row = const.tile([1, F1], bf16)
            nc.sync.dma_start(out=b1_row[:, :], in_=b1_d[None, :])
            b2_row = const.tile([1, F2], bf16)
            nc.sync.dma_start(out=b2_row[:, :], in_=b2_d[None, :])

            iota_i = const.tile([P, P], i32)
            nc.gpsimd.iota(iota_i[:], pattern=[[1, P]], base=0,
                           channel_multiplier=0)
            iota_bf = const.tile([P, P], bf16)
            nc.vector.tensor_copy(out=iota_bf[:], in_=iota_i[:])
            ident = const.tile([P, P], bf16)
            make_identity(nc, ident[:])

            idxlo_sb = const.tile([P, lo_cols], i16)
            nc.sync.dma_start(out=idxlo_sb[:], in_=idxlo_d[:])
            idxhi_sb = const.tile([P, hi_cols], i16)
            nc.sync.dma_start(out=idxhi_sb[:], in_=idxhi_d[:])
            dstl_sb = const.tile([P, total], bf16)
            nc.sync.dma_start(out=dstl_sb[:], in_=dstl_d[:])
            dis_sb = const.tile([P, ntiles], f32)
            nc.sync.dma_start(out=dis_sb[:], in_=dis_d[:])
            invdis_sb = const.tile([1, RP], bf16)
            nc.sync.dma_start(out=invdis_sb[:], in_=invdis_d[:])

            h1T = const.tile([P, H1 * RP], bf16)

            # ---------------- phase 1: xw1' = dis * (x_c @ W1) --------------
            with (
                tc.tile_pool(name="p1x", bufs=1) as p1x,
                tc.tile_pool(name="p1o", bufs=3) as p1o,
                tc.tile_pool(name="p1ps", bufs=2, space="PSUM") as p1ps,
            ):
                xt_sb = p1x.tile([P, K0 * R], bf16)
                nc.sync.dma_start(
                    out=xt_sb[:].rearrange("p (k r) -> p k r", k=K0),
                    in_=xT.rearrange("(k p) r -> p k r", p=P))
                for m in range(ntiles):
                    rows = last_rows if m == ntiles - 1 else P
                    ps = p1ps.tile([P, F1], f32)
                    for k in range(K0):
                        nc.tensor.matmul(
                            out=ps[:rows, :],
                            lhsT=xt_sb[:, k * R + m * P: k * R + m * P + rows],
                            rhs=w1_sb[:, k * F1:(k + 1) * F1],
                            start=(k == 0), stop=(k == K0 - 1))
                    os = p1o.tile([P, F1], bf16)
                    nc.scalar.activation(out=os[:rows, :], in_=ps[:rows, :],
                                         func=mybir.ActivationFunctionType.Copy,
                                         scale=dis_sb[:rows, m:m + 1])
                    nc.sync.dma_start(out=ag1_in[m * P: m * P + rows, :],
                                      in_=os[:rows, :])

            nc.gpsimd.collective_compute(
                "AllGather", mybir.AluOpType.bypass, replica_groups=rg,
                ins=[ag1_in[:].opt()], outs=[ag1_out[:].opt()])

            # ------- phase 2: aggregate layer 1 (node-major), then
            #         relu+bias+dis and PE-transpose into h1T -------
            def aggregate(t, ag_out, G_pool, S_pool, ps_pool, F):
                """Emit gathers + S build + matmul chain for tile t.
                Returns the psum tile (chain left open for the bias matmul)."""
                nch = chunks[t]
                j0 = offs[t]
                ps = ps_pool.tile([P, F], f32, tag="ps")
                G = G_pool.tile([P, maxch * F], bf16, tag="G")
                nlo, nhi = clo[t], chi[t]
                if nlo:
                    nc.gpsimd.dma_gather(
                        out_ap=G[:, :nlo * F].rearrange(
                            "p (c f) -> p c f", c=nlo),
                        in_ap=ag_out[:SPLIT, :],
                        idxs_ap=idxlo_sb[:, lo_offs[t] * 8:lo_offs[t + 1] * 8],
                        num_idxs=nlo * P, num_idxs_reg=nlo * P,
                        elem_size=F, single_packet=False)
                if nhi:
                    nc.gpsimd.dma_gather(
                        out_ap=G[:, nlo * F:nch * F].rearrange(
                            "p (c f) -> p c f", c=nhi),
                        in_ap=ag_out[SPLIT:, :],
                        idxs_ap=idxhi_sb[:, hi_offs[t] * 8:hi_offs[t + 1] * 8],
                        num_idxs=nhi * P, num_idxs_reg=nhi * P,
                        elem_size=F, single_packet=False)
                S = S_pool.tile([P, maxch * P], bf16, tag="S")
                nc.vector.tensor_tensor(
                    out=S[:, :nch * P].rearrange("p (c e) -> p c e", c=nch),
                    in0=iota_bf[:, None, :].to_broadcast([P, nch, P]),
                    in1=dstl_sb[:, j0:j0 + nch][:, :, None]
                        .to_broadcast([P, nch, P]),
                    op=mybir.AluOpType.is_equal)
                for c in range(nch):
                    nc.tensor.matmul(
                        out=ps[:], lhsT=S[:, c * P:(c + 1) * P],
                        rhs=G[:, c * F:(c + 1) * F],
                        start=(c == 0), stop=False)
                return ps, nch

            with (
                tc.tile_pool(name="p2g", bufs=2) as p2g,
                tc.tile_pool(name="p2s", bufs=2) as p2s,
                tc.tile_pool(name="p2h", bufs=3) as p2h,
                tc.tile_pool(name="p2ps", bufs=2, space="PSUM") as p2ps,
                tc.tile_pool(name="p2pt", bufs=3, space="PSUM") as p2pt,
            ):
                for t in range(ntiles):
                    ps, nch = aggregate(t, ag1_out, p2g, p2s, p2ps, F1)
                    nc.tensor.matmul(out=ps[:],
                                     lhsT=invdis_sb[:, t * P:(t + 1) * P],
                                     rhs=b1_row[:], start=(nch == 0),
                                     stop=True)
                    hm = p2h.tile([P, F1], bf16, tag="hm")
                    nc.scalar.activation(
                        out=hm[:], in_=ps[:],
                        func=mybir.ActivationFunctionType.Relu,
                        scale=dis_sb[:, t:t + 1])
                    for h in range(H1):
                        pt = p2pt.tile([P, P], bf16, tag="pt")
                        nc.tensor.transpose(
                            out=pt[:], in_=hm[:, h * P:(h + 1) * P],
                            identity=ident[:])
                        nc.vector.tensor_copy(
                            out=h1T[:, h * RP + t * P: h * RP + (t + 1) * P],
                            in_=pt[:])

            # ---------------- phase 3: hw2' = dis * (h1 @ W2) ---------------
            with (
                tc.tile_pool(name="p3o", bufs=3) as p3o,
                tc.tile_pool(name="p3ps", bufs=2, space="PSUM") as p3ps,
            ):
                for m in range(ntiles):
                    rows = last_rows if m == ntiles - 1 else P
                    ps = p3ps.tile([P, F2], f32)
                    for k in range(K2):
                        nc.tensor.matmul(
                            out=ps[:rows, :],
                            lhsT=h1T[:, k * RP + m * P: k * RP + m * P + rows],
                            rhs=w2_sb[:, k * F2:(k + 1) * F2],
                            start=(k == 0), stop=(k == K2 - 1))
                    os = p3o.tile([P, F2], bf16)
                    nc.scalar.activation(out=os[:rows, :], in_=ps[:rows, :],
                                         func=mybir.ActivationFunctionType.Copy,
                                         scale=dis_sb[:rows, m:m + 1])
                    nc.sync.dma_start(out=ag2_in[m * P: m * P + rows, :],
                                      in_=os[:rows, :])

            nc.gpsimd.collective_compute(
                "AllGather", mybir.AluOpType.bypass, replica_groups=rg,
                ins=[ag2_in[:].opt()], outs=[ag2_out[:].opt()])

            # ------- phase 4: aggregate layer 2, node-major out -------
            with (
                tc.tile_pool(name="p4g", bufs=2) as p4g,
                tc.tile_pool(name="p4s", bufs=2) as p4s,
                tc.tile_pool(name="p4o", bufs=3) as p4o,
                tc.tile_pool(name="p4ps", bufs=2, space="PSUM") as p4ps,
            ):
                for t in range(ntiles):
                    rows = last_rows if t == ntiles - 1 else P
                    ps, nch = aggregate(t, ag2_out, p4g, p4s, p4ps, F2)
                    nc.tensor.matmul(out=ps[:],
                                     lhsT=invdis_sb[:, t * P:(t + 1) * P],
                                     rhs=b2_row[:], start=(nch == 0),
                                     stop=True)
                    os = p4o.tile([P, F2], bf16)
                    nc.scalar.activation(out=os[:rows, :], in_=ps[:rows, :],
                                         func=mybir.ActivationFunctionType.Copy,
                                         scale=dis_sb[:rows, t:t + 1])
                    nc.sync.dma_start(out=out_d[t * P: t * P + rows, :],
                                      in_=os[:rows, :])

    nc.compile()
    return nc


# ----------------------------------------------------------------------------
# Public entry point
# ----------------------------------------------------------------------------

LAST_EXEC_NS = None
LAST_RESULTS = None


def kernel(x, edge_index, W1, b1, W2, b2, _trace=False, _tmpdir=None):
    global LAST_EXEC_NS, LAST_RESULTS
    x = np.asarray(x, np.float32)
    edge_index = np.asarray(edge_index)
    W1 = np.asarray(W1, np.float32)
    b1 = np.asarray(b1, np.float32)
    W2 = np.asarray(W2, np.float32)
    b2 = np.asarray(b2, np.float32)
    N, F0 = x.shape
    F1 = W1.shape[1]
    F2 = W2.shape[1]

    packed, clo, chi, R, ntiles = _preprocess(x, edge_index, NCORES)
    nc = build_nc(N, R, ntiles, clo, chi, F0, F1, F2, NCORES)

    W1b = W1.astype(BF16)
    W2b = W2.astype(BF16)
    b1b = b1.astype(BF16)
    b2b = b2.astype(BF16)
    in_maps = []
    for c in range(NCORES):
        idx_lo, idx_hi, d_a, dis_t, invdis_r = packed[c]
        xT_c = np.ascontiguousarray(x[c * R:(c + 1) * R].T).astype(BF16)
        in_maps.append({
            "xT": xT_c, "idxlo": idx_lo, "idxhi": idx_hi, "dstl": d_a,
            "dis": dis_t, "invdis": invdis_r,
            "W1": W1b, "b1": b1b, "W2": W2b, "b2": b2b,
        })

    res = bass_utils.run_bass_kernel_spmd(
        nc, in_maps, core_ids=list(range(NCORES)), trace=_trace,
        tmpdir=_tmpdir)
    LAST_EXEC_NS = res.exec_time_ns
    LAST_RESULTS = res
    out = np.concatenate([res.results[c]["out"] for c in range(NCORES)], axis=0)
    return out.astype(np.float32)


# revision 12
# speedup vs baseline: 1.1817x; 1.1817x over previous
"""2-layer GCN (PyG GCNConv style) on 8 Trainium2 NeuronCores.

Strategy (graph/node parallel, per sharding hint):
  - Nodes are range-sharded across 8 cores (R = N/8 rows each).
  - All device compute runs in bf16 (fp32 PSUM accumulation); tolerance
    is 2e-2 so bf16 is plenty: 4x PE rate + FWL weight loads + half the
    DMA/collective bytes.
  - Normalization factoring: with dis = deg^-1/2,
        out = Dis . A01^T . Dis . (x @ W) + b
    so the per-edge norm never appears on device:
      * dis[src] is folded into the gathered rows (phase-1/3 PSUM->SBUF
        copy is an activation with per-partition scale=dis).
      * dis[dst] is applied on the aggregated output tile (activation
        scale), with the bias pre-divided via a rank-1 matmul
        lhsT=invdis so that relu((agg + b/dis)*dis) == relu(agg*dis+b).
      * The selection matrix S becomes a pure 0/1 one-hot built with a
        single batched DVE is_equal per destination tile.
  - Per dst-tile (128 nodes), edges are packed into chunks of 128 and
    gathered with TWO dma_gather calls per tile (sources < 32768 from
    the table base, sources >= 32768 from a shifted base, because
    dma_gather indices are int16).  One dma_gather amortizes the ~1us
    SWDGE fixed cost over a whole tile's rows.  Slot i of the index
    list lands at out[i % 128, i // 128, :]; pad slots use index 0 and
    dstl=255 so S zeroes their contribution.
  - Device per core:
      phase 1: xw1' = dis_c * (x_c @ W1)            -> AllGather (bf16)
      phase 2: aggregate -> relu((agg + b1/dis)*dis) -> h1, PE-transpose
               to h1T in SBUF
      phase 3: hw2' = dis_c * (h1 @ W2)             -> AllGather (bf16)
      phase 4: aggregate -> (agg + b2/dis)*dis      -> out (bf16)
  - Host concatenates the 8 row-shards and casts to f32.
"""

import sys

for p in ("/opt/trn_rl_repo",):
    if p not in sys.path:
        sys.path.insert(0, p)

import numpy as np
import ml_dtypes

import concourse.bass as bass
import concourse.bacc as bacc
import concourse.mybir as mybir
import concourse.tile as tile
from concourse import bass_utils
from concourse.masks import make_identity

P = 128
NCORES = 8
BF16 = ml_dtypes.bfloat16
SPLIT = 32768  # int16 index limit for dma_gather


# ----------------------------------------------------------------------------
# Host-side preprocessing
# ----------------------------------------------------------------------------

def _preprocess(x, edge_index, n_cores):
    """Pack per-core edge data for the LO/HI dma_gather scheme.

    Returns per-core tuples (idx16, dstl, dis_tiles, invdis_row) plus the
    shared chunk structure (clo, chi) per tile.
    """
    N = x.shape[0]
    R = N // n_cores
    assert R * n_cores == N
    ntiles = (R + P - 1) // P
    RP = ntiles * P

    src = edge_index[0].astype(np.int64)
    dst = edge_index[1].astype(np.int64)
    loops = np.arange(N, dtype=np.int64)
    src = np.concatenate([src, loops])
    dst = np.concatenate([dst, loops])

    deg = np.bincount(dst, minlength=N).astype(np.float32)
    dis = np.where(deg > 0, 1.0 / np.sqrt(deg), 0.0).astype(np.float32)
    invdis = np.where(deg > 0, np.sqrt(deg), 0.0).astype(np.float32)

    core_id = dst // R
    dloc = dst - core_id * R
    tl = dloc // P
    dstl = (dloc - tl * P).astype(np.float32)
    is_hi = (src >= SPLIT).astype(np.int64)

    # per (core, tile): lo and hi edge counts -> shared chunk counts
    key = (core_id * ntiles + tl) * 2 + is_hi
    counts = np.bincount(key, minlength=n_cores * ntiles * 2)
    counts = counts.reshape(n_cores, ntiles, 2)
    clo = np.ceil(counts[:, :, 0].max(axis=0) / P).astype(np.int64)
    chi = np.ceil(counts[:, :, 1].max(axis=0) / P).astype(np.int64)
    chunks = clo + chi                     # chunks per tile (shared)
    total = int(chunks.sum())
    offs = np.concatenate([[0], np.cumsum(chunks)])      # chunk offsets
    lo_slots = int(clo.sum()) * P          # total LO slot count
    hi_slots = int(chi.sum()) * P
    lo_offs = np.concatenate([[0], np.cumsum(clo)])      # in chunks
    hi_offs = np.concatenate([[0], np.cumsum(chi)])

    packed = []
    for c in range(n_cores):
        m = core_id == c
        s_c = src[m]
        t_c = tl[m]
        d_c = dstl[m]
        h_c = is_hi[m]
        # order edges by (tile, hi, arbitrary)
        order = np.lexsort((h_c, t_c))
        s_c, t_c, d_c, h_c = s_c[order], t_c[order], d_c[order], h_c[order]

        # slot position within the tile's lo/hi group
        grp = t_c * 2 + h_c
        cnt = np.bincount(grp, minlength=ntiles * 2)
        starts = np.cumsum(cnt) - cnt
        pos = np.arange(len(s_c)) - np.repeat(starts, cnt)

        # dstl array: [P, total] with per-tile chunk order [LO..., HI...]
        A_dl = np.full(total * P, 255.0, np.float32)
        chunk_in_tile = np.where(h_c == 0, pos // P, clo[t_c] + pos // P)
        slots = (offs[t_c] + chunk_in_tile) * P + pos % P
        A_dl[slots] = d_c

        # index lists (flat slot order: position i -> chunk i//128, lane i%128)
        idx_lo = np.zeros(lo_slots, np.int16)
        idx_hi = np.zeros(hi_slots, np.int16)
        mlo = h_c == 0
        idx_lo[(lo_offs[t_c[mlo]] * P + pos[mlo]).astype(np.int64)] = \
            s_c[mlo].astype(np.int16)
        mhi = h_c == 1
        idx_hi[(hi_offs[t_c[mhi]] * P + pos[mhi]).astype(np.int64)] = \
            (s_c[mhi] - SPLIT).astype(np.int16)

        def lay_dl(a):
            return np.ascontiguousarray(a.reshape(total, P).T).astype(BF16)

        def lay_idx(a):
            # [16, n/16] wrapped (list pos j -> [j%16, j//16]), replicated
            # across the 128 partitions (each Q7 core reads its own window)
            w = np.ascontiguousarray(a.reshape(-1, 16).T)  # [16, n/16]
            return np.ascontiguousarray(np.tile(w, (8, 1)))

        dis_c = np.zeros(RP, np.float32)
        dis_c[:R] = dis[c * R:(c + 1) * R]
        invdis_c = np.zeros(RP, np.float32)
        invdis_c[:R] = invdis[c * R:(c + 1) * R]
        dis_tiles = np.ascontiguousarray(dis_c.reshape(ntiles, P).T)
        invdis_row = invdis_c.reshape(1, RP).astype(BF16)

        packed.append((lay_idx(idx_lo), lay_idx(idx_hi), lay_dl(A_dl),
                       dis_tiles, invdis_row))
    return (packed, [int(v) for v in clo], [int(v) for v in chi], R, ntiles)


# ----------------------------------------------------------------------------
# Device kernel builder
# ----------------------------------------------------------------------------

def build_nc(N, R, ntiles, clo, chi, F0, F1, F2, n_cores):
    """Build the SPMD Bass program. All dims: F0,F1,F2 multiples of 128."""
    f32 = mybir.dt.float32
    bf16 = mybir.dt.bfloat16
    i16 = mybir.dt.int16
    i32 = mybir.dt.int32
    K0 = F0 // P       # k-tiles in layer-1 matmul
    H1 = F1 // P       # 128-wide halves of F1
    K2 = F1 // P       # k-tiles in layer-2 matmul (= H1)
    assert F2 <= 512 and F2 % P == 0
    last_rows = R - (ntiles - 1) * P
    RP = ntiles * P
    chunks = [a + b for a, b in zip(clo, chi)]
    total = int(sum(chunks))
    maxch = max(chunks)
    offs = [0]
    for c in chunks:
        offs.append(offs[-1] + c)
    lo_offs = [0]
    for c in clo:
        lo_offs.append(lo_offs[-1] + c)
    hi_offs = [0]
    for c in chi:
        hi_offs.append(hi_offs[-1] + c)
    lo_cols = lo_offs[-1] * 8   # int16 idx columns ([16-row blocks] * 8 per chunk)
    hi_cols = hi_offs[-1] * 8

    nc = bacc.Bacc("TRN2", target_bir_lowering=False, debug=False,
                   num_devices=n_cores, num_swdge_queues=4)

    xT = nc.dram_tensor("xT", [F0, R], bf16, kind="ExternalInput").ap()
    idxlo_d = nc.dram_tensor("idxlo", [P, lo_cols], i16,
                             kind="ExternalInput").ap()
    idxhi_d = nc.dram_tensor("idxhi", [P, hi_cols], i16,
                             kind="ExternalInput").ap()
    dstl_d = nc.dram_tensor("dstl", [P, total], bf16,
                            kind="ExternalInput").ap()
    dis_d = nc.dram_tensor("dis", [P, ntiles], f32, kind="ExternalInput").ap()
    invdis_d = nc.dram_tensor("invdis", [1, RP], bf16,
                              kind="ExternalInput").ap()
    W1_d = nc.dram_tensor("W1", [F0, F1], bf16, kind="ExternalInput").ap()
    b1_d = nc.dram_tensor("b1", [F1], bf16, kind="ExternalInput").ap()
    W2_d = nc.dram_tensor("W2", [F1, F2], bf16, kind="ExternalInput").ap()
    b2_d = nc.dram_tensor("b2", [F2], bf16, kind="ExternalInput").ap()
    out_d = nc.dram_tensor("out", [R, F2], bf16, kind="ExternalOutput").ap()

    rg = [list(range(n_cores))]

    with tile.TileContext(nc) as tc:
        with (
            tc.tile_pool(name="dram", bufs=1, space="DRAM") as dram,
            tc.tile_pool(name="const", bufs=1) as const,
        ):
            ag1_in = dram.tile([R, F1], bf16)
            ag1_out = dram.tile([N, F1], bf16, addr_space="Shared")
            ag2_in = dram.tile([R, F2], bf16)
            ag2_out = dram.tile([N, F2], bf16, addr_space="Shared")

            w1_sb = const.tile([P, K0 * F1], bf16)
            nc.sync.dma_start(
                out=w1_sb[:].rearrange("p (k f) -> p k f", k=K0),
                in_=W1_d.rearrange("(k p) f -> p k f", p=P))
            w2_sb = const.tile([P, K2 * F2], bf16)
            nc.sync.dma_start(
                out=w2_sb[:].rearrange("p (k f) -> p k f", k=K2),
                in_=W2_d.rearrange("(k p) f -> p k f", p=P))
            b1_row = const.tile([1, F1], bf16)
            nc.sync.dma_start(out=b1_row[:, :], in_=b1_d[None, :])
            b2_row = const.tile([1, F2], bf16)
            nc.sync.dma_start(out=b2_row[:, :], in_=b2_d[None, :])

            iota_i = const.tile([P, P], i32)
            nc.gpsimd.iota(iota_i[:], pattern=[[1, P]], base=0,
                           channel_multiplier=0)
            iota_bf = const.tile([P, P], bf16)
            nc.vector.tensor_copy(out=iota_bf[:], in_=iota_i[:])
            ident = const.tile([P, P], bf16)
            make_identity(nc, ident[:])

            idxlo_sb = const.tile([P, lo_cols], i16)
            nc.sync.dma_start(out=idxlo_sb[:], in_=idxlo_d[:])
            idxhi_sb = const.tile([P, hi_cols], i16)
            nc.sync.dma_start(out=idxhi_sb[:], in_=idxhi_d[:])
            dstl_sb = const.tile([P, total], bf16)
            nc.sync.dma_start(out=dstl_sb[:], in_=dstl_d[:])
            dis_sb = const.tile([P, ntiles], f32)
            nc.sync.dma_start(out=dis_sb[:], in_=dis_d[:])
            invdis_sb = const.tile([1, RP], bf16)
            nc.sync.dma_start(out=invdis_sb[:], in_=invdis_d[:])

            h1T = const.tile([P, H1 * RP], bf16)

            # ---------------- phase 1: xw1' = dis * (x_c @ W1) --------------
            with (
                tc.tile_pool(name="p1x", bufs=1) as p1x,
                tc.tile_pool(name="p1o", bufs=3) as p1o,
                tc.tile_pool(name="p1ps", bufs=2, space="PSUM") as p1ps,
            ):
                xt_sb = p1x.tile([P, K0 * R], bf16)
                nc.sync.dma_start(
                    out=xt_sb[:].rearrange("p (k r) -> p k r", k=K0),
                    in_=xT.rearrange("(k p) r -> p k r", p=P))
                for m in range(ntiles):
                    rows = last_rows if m == ntiles - 1 else P
                    ps = p1ps.tile([P, F1], f32)
                    for k in range(K0):
                        nc.tensor.matmul(
                            out=ps[:rows, :],
                            lhsT=xt_sb[:, k * R + m * P: k * R + m * P + rows],
                            rhs=w1_sb[:, k * F1:(k + 1) * F1],
                            start=(k == 0), stop=(k == K0 - 1))
                    os = p1o.tile([P, F1], bf16)
                    nc.scalar.activation(out=os[:rows, :], in_=ps[:rows, :],
                                         func=mybir.ActivationFunctionType.Copy,
                                         scale=dis_sb[:rows, m:m + 1])
                    nc.sync.dma_start(out=ag1_in[m * P: m * P + rows, :],
                                      in_=os[:rows, :])

            nc.gpsimd.collective_compute(
                "AllGather", mybir.AluOpType.bypass, replica_groups=rg,
                ins=[ag1_in[:].opt()], outs=[ag1_out[:].opt()])

            # ------- phase 2: aggregate layer 1 (node-major), then
            #         relu+bias+dis and PE-transpose into h1T -------
            qctr = [0]

            def aggregate(t, ag_out, G_pool, S_pool, ps_pool, F):
                """Emit gathers + S build + matmul chain for tile t.
                Returns the psum tile (chain left open for the bias matmul)."""
                nch = chunks[t]
                j0 = offs[t]
                ps = ps_pool.tile([P, F], f32, tag="ps")
                G = G_pool.tile([P, maxch * F], bf16, tag="G")
                nlo, nhi = clo[t], chi[t]
                if nlo:
                    nc.gpsimd.dma_gather(
                        out_ap=G[:, :nlo * F].rearrange(
                            "p (c f) -> p c f", c=nlo),
                        in_ap=ag_out[:SPLIT, :],
                        idxs_ap=idxlo_sb[:, lo_offs[t] * 8:lo_offs[t + 1] * 8],
                        num_idxs=nlo * P, num_idxs_reg=nlo * P,
                        elem_size=F, single_packet=False,
                        queue_num=qctr[0] % 4)
                    qctr[0] += 1
                if nhi:
                    nc.gpsimd.dma_gather(
                        out_ap=G[:, nlo * F:nch * F].rearrange(
                            "p (c f) -> p c f", c=nhi),
                        in_ap=ag_out[SPLIT:, :],
                        idxs_ap=idxhi_sb[:, hi_offs[t] * 8:hi_offs[t + 1] * 8],
                        num_idxs=nhi * P, num_idxs_reg=nhi * P,
                        elem_size=F, single_packet=False,
                        queue_num=qctr[0] % 4)
                    qctr[0] += 1
                S = S_pool.tile([P, maxch * P], bf16, tag="S")
                nc.vector.tensor_tensor(
                    out=S[:, :nch * P].rearrange("p (c e) -> p c e", c=nch),
                    in0=iota_bf[:, None, :].to_broadcast([P, nch, P]),
                    in1=dstl_sb[:, j0:j0 + nch][:, :, None]
                        .to_broadcast([P, nch, P]),
                    op=mybir.AluOpType.is_equal)
                for c in range(nch):
                    nc.tensor.matmul(
                        out=ps[:], lhsT=S[:, c * P:(c + 1) * P],
                        rhs=G[:, c * F:(c + 1) * F],
                        start=(c == 0), stop=False)
                return ps, nch

            # phase 3 (hw2' = dis * (h1 @ W2)) is interleaved per tile right
            # after the tile's h1T is available, so it hides entirely under
            # the phase-2 gather stream and AG2 can fire the moment phase 2
            # ends.
            with (
                tc.tile_pool(name="p2g", bufs=2) as p2g,
                tc.tile_pool(name="p2s", bufs=2) as p2s,
                tc.tile_pool(name="p2h", bufs=3) as p2h,
                tc.tile_pool(name="p2ps", bufs=2, space="PSUM") as p2ps,
                tc.tile_pool(name="p2pt", bufs=3, space="PSUM") as p2pt,
                tc.tile_pool(name="p3o", bufs=3) as p3o,
                tc.tile_pool(name="p3ps", bufs=2, space="PSUM") as p3ps,
            ):
                for t in range(ntiles):
                    rows = last_rows if t == ntiles - 1 else P
                    ps, nch = aggregate(t, ag1_out, p2g, p2s, p2ps, F1)
                    nc.tensor.matmul(out=ps[:],
                                     lhsT=invdis_sb[:, t * P:(t + 1) * P],
                                     rhs=b1_row[:], start=(nch == 0),
                                     stop=True)
                    hm = p2h.tile([P, F1], bf16, tag="hm")
                    nc.scalar.activation(
                        out=hm[:], in_=ps[:],
                        func=mybir.ActivationFunctionType.Relu,
                        scale=dis_sb[:, t:t + 1])
                    for h in range(H1):
                        pt = p2pt.tile([P, P], bf16, tag="pt")
                        nc.tensor.transpose(
                            out=pt[:], in_=hm[:, h * P:(h + 1) * P],
                            identity=ident[:])
                        nc.vector.tensor_copy(
                            out=h1T[:, h * RP + t * P: h * RP + (t + 1) * P],
                            in_=pt[:])
                    # phase-3 matmul for this tile
                    ps3 = p3ps.tile([P, F2], f32, tag="ps3")
                    for k in range(K2):
                        nc.tensor.matmul(
                            out=ps3[:rows, :],
                            lhsT=h1T[:, k * RP + t * P: k * RP + t * P + rows],
                            rhs=w2_sb[:, k * F2:(k + 1) * F2],
                            start=(k == 0), stop=(k == K2 - 1))
                    os3 = p3o.tile([P, F2], bf16, tag="os3")
                    nc.scalar.activation(out=os3[:rows, :], in_=ps3[:rows, :],
                                         func=mybir.ActivationFunctionType.Copy,
                                         scale=dis_sb[:rows, t:t + 1])
                    nc.sync.dma_start(out=ag2_in[t * P: t * P + rows, :],
                                      in_=os3[:rows, :])

            nc.gpsimd.collective_compute(
                "AllGather", mybir.AluOpType.bypass, replica_groups=rg,
                ins=[ag2_in[:].opt()], outs=[ag2_out[:].opt()])

            # ------- phase 4: aggregate layer 2, node-major out -------
            with (
                tc.tile_pool(name="p4g", bufs=2) as p4g,
                tc.tile_pool(name="p4s", bufs=2) as p4s,
                tc.tile_pool(name="p4o", bufs=3) as p4o,
                tc.tile_pool(name="p4ps", bufs=2, space="PSUM") as p4ps,
            ):
                for t in range(ntiles):
                    rows = last_rows if t == ntiles - 1 else P
                    ps, nch = aggregate(t, ag2_out, p4g, p4s, p4ps, F2)
                    nc.tensor.matmul(out=ps[:],
                                     lhsT=invdis_sb[:, t * P:(t + 1) * P],
                                     rhs=b2_row[:], start=(nch == 0),
                                     stop=True)
                    os = p4o.tile([P, F2], bf16)
                    nc.scalar.activation(out=os[:rows, :], in_=ps[:rows, :],
                                         func=mybir.ActivationFunctionType.Copy,
                                         scale=dis_sb[:rows, t:t + 1])
                    nc.sync.dma_start(out=out_d[t * P: t * P + rows, :],
                                      in_=os[:rows, :])

    nc.compile()
    return nc


# ----------------------------------------------------------------------------
# Public entry point
# ----------------------------------------------------------------------------

LAST_EXEC_NS = None
LAST_RESULTS = None


def kernel(x, edge_index, W1, b1, W2, b2, _trace=False, _tmpdir=None):
    global LAST_EXEC_NS, LAST_RESULTS
    x = np.asarray(x, np.float32)
    edge_index = np.asarray(edge_index)
    W1 = np.asarray(W1, np.float32)
    b1 = np.asarray(b1, np.float32)
    W2 = np.asarray(W2, np.float32)
    b2 = np.asarray(b2, np.float32)
    N, F0 = x.shape
    F1 = W1.shape[1]
    F2 = W2.shape[1]

    packed, clo, chi, R, ntiles = _preprocess(x, edge_index, NCORES)
    nc = build_nc(N, R, ntiles, clo, chi, F0, F1, F2, NCORES)

    W1b = W1.astype(BF16)
    W2b = W2.astype(BF16)
    b1b = b1.astype(BF16)
    b2b = b2.astype(BF16)
    in_maps = []
    for c in range(NCORES):
        idx_lo, idx_hi, d_a, dis_t, invdis_r = packed[c]
        xT_c = np.ascontiguousarray(x[c * R:(c + 1) * R].T).astype(BF16)
        in_maps.append({
            "xT": xT_c, "idxlo": idx_lo, "idxhi": idx_hi, "dstl": d_a,
            "dis": dis_t, "invdis": invdis_r,
            "W1": W1b, "b1": b1b, "W2": W2b, "b2": b2b,
        })

    res = bass_utils.run_bass_kernel_spmd(
        nc, in_maps, core_ids=list(range(NCORES)), trace=_trace,
        tmpdir=_tmpdir)
    LAST_EXEC_NS = res.exec_time_ns
    LAST_RESULTS = res
    out = np.concatenate([res.results[c]["out"] for c in range(NCORES)], axis=0)
    return out.astype(np.float32)


# revision 13
# speedup vs baseline: 1.4716x; 1.2454x over previous
"""2-layer GCN (PyG GCNConv style) on 8 Trainium2 NeuronCores.

Strategy (graph/node parallel, per sharding hint):
  - Nodes are range-sharded across 8 cores (R = N/8 rows each).
  - All device compute runs in bf16 (fp32 PSUM accumulation); tolerance
    is 2e-2 so bf16 is plenty: 4x PE rate + FWL weight loads + half the
    DMA/collective bytes.
  - Normalization factoring: with dis = deg^-1/2,
        out = Dis . A01^T . Dis . (x @ W) + b
    so the per-edge norm never appears on device:
      * dis[src] is folded into the gathered rows (phase-1/3 PSUM->SBUF
        copy is an activation with per-partition scale=dis).
      * dis[dst] is applied on the aggregated output tile (activation
        scale), with the bias pre-divided via a rank-1 matmul
        lhsT=invdis so that relu((agg + b/dis)*dis) == relu(agg*dis+b).
      * The selection matrix S becomes a pure 0/1 one-hot built with a
        single batched DVE is_equal per destination tile.
  - Per dst-tile (128 nodes), edges are packed into chunks of 128 and
    gathered with TWO dma_gather calls per tile (sources < 32768 from
    the table base, sources >= 32768 from a shifted base, because
    dma_gather indices are int16).  One dma_gather amortizes the ~1us
    SWDGE fixed cost over a whole tile's rows.  Slot i of the index
    list lands at out[i % 128, i // 128, :]; pad slots use index 0 and
    dstl=255 so S zeroes their contribution.
  - Device per core:
      phase 1: xw1' = dis_c * (x_c @ W1)            -> AllGather (bf16)
      phase 2: aggregate -> relu((agg + b1/dis)*dis) -> h1, PE-transpose
               to h1T in SBUF
      phase 3: hw2' = dis_c * (h1 @ W2)             -> AllGather (bf16)
      phase 4: aggregate -> (agg + b2/dis)*dis      -> out (bf16)
  - Host concatenates the 8 row-shards and casts to f32.
"""

import sys

for p in ("/opt/trn_rl_repo",):
    if p not in sys.path:
        sys.path.insert(0, p)

import numpy as np
import ml_dtypes

import concourse.bass as bass
import concourse.bacc as bacc
import concourse.mybir as mybir
import concourse.tile as tile
from concourse import bass_utils
from concourse.masks import make_identity

P = 128
NCORES = 8
BF16 = ml_dtypes.bfloat16
SPLIT = 32768  # int16 index limit for dma_gather


# ----------------------------------------------------------------------------
# Host-side preprocessing
# ----------------------------------------------------------------------------

def _preprocess(x, edge_index, n_cores):
    """Pack per-core edge data for the LO/HI dma_gather scheme.

    Returns per-core tuples (idx16, dstl, dis_tiles, invdis_row) plus the
    shared chunk structure (clo, chi) per tile.
    """
    N = x.shape[0]
    R = N // n_cores
    assert R * n_cores == N
    ntiles = (R + P - 1) // P
    RP = ntiles * P

    src = edge_index[0].astype(np.int64)
    dst = edge_index[1].astype(np.int64)
    loops = np.arange(N, dtype=np.int64)
    src = np.concatenate([src, loops])
    dst = np.concatenate([dst, loops])

    deg = np.bincount(dst, minlength=N).astype(np.float32)
    dis = np.where(deg > 0, 1.0 / np.sqrt(deg), 0.0).astype(np.float32)
    invdis = np.where(deg > 0, np.sqrt(deg), 0.0).astype(np.float32)

    core_id = dst // R
    dloc = dst - core_id * R
    tl = dloc // P
    dstl = (dloc - tl * P).astype(np.float32)
    is_hi = (src >= SPLIT).astype(np.int64)

    # per (core, tile): lo and hi edge counts -> shared chunk counts
    key = (core_id * ntiles + tl) * 2 + is_hi
    counts = np.bincount(key, minlength=n_cores * ntiles * 2)
    counts = counts.reshape(n_cores, ntiles, 2)
    clo = np.ceil(counts[:, :, 0].max(axis=0) / P).astype(np.int64)
    chi = np.ceil(counts[:, :, 1].max(axis=0) / P).astype(np.int64)
    chunks = clo + chi                     # chunks per tile (shared)
    total = int(chunks.sum())
    offs = np.concatenate([[0], np.cumsum(chunks)])      # chunk offsets
    lo_slots = int(clo.sum()) * P          # total LO slot count
    hi_slots = int(chi.sum()) * P
    lo_offs = np.concatenate([[0], np.cumsum(clo)])      # in chunks
    hi_offs = np.concatenate([[0], np.cumsum(chi)])

    packed = []
    for c in range(n_cores):
        m = core_id == c
        s_c = src[m]
        t_c = tl[m]
        d_c = dstl[m]
        h_c = is_hi[m]
        # order edges by (tile, hi, arbitrary)
        order = np.lexsort((h_c, t_c))
        s_c, t_c, d_c, h_c = s_c[order], t_c[order], d_c[order], h_c[order]

        # slot position within the tile's lo/hi group
        grp = t_c * 2 + h_c
        cnt = np.bincount(grp, minlength=ntiles * 2)
        starts = np.cumsum(cnt) - cnt
        pos = np.arange(len(s_c)) - np.repeat(starts, cnt)

        # dstl array: [P, total] with per-tile chunk order [LO..., HI...]
        A_dl = np.full(total * P, 255.0, np.float32)
        chunk_in_tile = np.where(h_c == 0, pos // P, clo[t_c] + pos // P)
        slots = (offs[t_c] + chunk_in_tile) * P + pos % P
        A_dl[slots] = d_c

        # index lists (flat slot order: position i -> chunk i//128, lane i%128)
        idx_lo = np.zeros(lo_slots, np.int16)
        idx_hi = np.zeros(hi_slots, np.int16)
        mlo = h_c == 0
        idx_lo[(lo_offs[t_c[mlo]] * P + pos[mlo]).astype(np.int64)] = \
            s_c[mlo].astype(np.int16)
        mhi = h_c == 1
        idx_hi[(hi_offs[t_c[mhi]] * P + pos[mhi]).astype(np.int64)] = \
            (s_c[mhi] - SPLIT).astype(np.int16)

        def lay_dl(a):
            return np.ascontiguousarray(a.reshape(total, P).T).astype(BF16)

        def lay_idx(a):
            # [16, n/16] wrapped (list pos j -> [j%16, j//16]), replicated
            # across the 128 partitions (each Q7 core reads its own window)
            w = np.ascontiguousarray(a.reshape(-1, 16).T)  # [16, n/16]
            return np.ascontiguousarray(np.tile(w, (8, 1)))

        dis_c = np.zeros(RP, np.float32)
        dis_c[:R] = dis[c * R:(c + 1) * R]
        invdis_c = np.zeros(RP, np.float32)
        invdis_c[:R] = invdis[c * R:(c + 1) * R]
        dis_tiles = np.ascontiguousarray(dis_c.reshape(ntiles, P).T)
        invdis_row = invdis_c.reshape(1, RP).astype(BF16)

        packed.append((lay_idx(idx_lo), lay_idx(idx_hi), lay_dl(A_dl),
                       dis_tiles, invdis_row))
    return (packed, [int(v) for v in clo], [int(v) for v in chi], R, ntiles)


# ----------------------------------------------------------------------------
# Device kernel builder
# ----------------------------------------------------------------------------

def build_nc(N, R, ntiles, clo, chi, F0, F1, F2, n_cores):
    """Build the SPMD Bass program. All dims: F0,F1,F2 multiples of 128."""
    f32 = mybir.dt.float32
    bf16 = mybir.dt.bfloat16
    i16 = mybir.dt.int16
    i32 = mybir.dt.int32
    K0 = F0 // P       # k-tiles in layer-1 matmul
    H1 = F1 // P       # 128-wide halves of F1
    K2 = F1 // P       # k-tiles in layer-2 matmul (= H1)
    assert F2 <= 512 and F2 % P == 0
    last_rows = R - (ntiles - 1) * P
    RP = ntiles * P
    chunks = [a + b for a, b in zip(clo, chi)]
    total = int(sum(chunks))
    maxch = max(chunks)
    offs = [0]
    for c in chunks:
        offs.append(offs[-1] + c)
    lo_offs = [0]
    for c in clo:
        lo_offs.append(lo_offs[-1] + c)
    hi_offs = [0]
    for c in chi:
        hi_offs.append(hi_offs[-1] + c)
    lo_cols = lo_offs[-1] * 8   # int16 idx columns ([16-row blocks] * 8 per chunk)
    hi_cols = hi_offs[-1] * 8

    nc = bacc.Bacc("TRN2", target_bir_lowering=False, debug=False,
                   num_devices=n_cores, num_swdge_queues=4)

    xT = nc.dram_tensor("xT", [F0, R], bf16, kind="ExternalInput").ap()
    idxlo_d = nc.dram_tensor("idxlo", [P, lo_cols], i16,
                             kind="ExternalInput").ap()
    idxhi_d = nc.dram_tensor("idxhi", [P, hi_cols], i16,
                             kind="ExternalInput").ap()
    dstl_d = nc.dram_tensor("dstl", [P, total], bf16,
                            kind="ExternalInput").ap()
    dis_d = nc.dram_tensor("dis", [P, ntiles], f32, kind="ExternalInput").ap()
    invdis_d = nc.dram_tensor("invdis", [1, RP], bf16,
                              kind="ExternalInput").ap()
    W1_d = nc.dram_tensor("W1", [F0, F1], bf16, kind="ExternalInput").ap()
    b1_d = nc.dram_tensor("b1", [F1], bf16, kind="ExternalInput").ap()
    W2_d = nc.dram_tensor("W2", [F1, F2], bf16, kind="ExternalInput").ap()
    b2_d = nc.dram_tensor("b2", [F2], bf16, kind="ExternalInput").ap()
    out_d = nc.dram_tensor("out", [R, F2], bf16, kind="ExternalOutput").ap()

    rg = [list(range(n_cores))]

    with tile.TileContext(nc) as tc:
        with (
            tc.tile_pool(name="dram", bufs=1, space="DRAM") as dram,
            tc.tile_pool(name="const", bufs=1) as const,
        ):
            ag1_in = dram.tile([R, F1], bf16)
            ag1_out = dram.tile([N, F1], bf16, addr_space="Shared")
            ag2_in = dram.tile([R, F2], bf16)
            ag2_out = dram.tile([N, F2], bf16, addr_space="Shared")

            w1_sb = const.tile([P, K0 * F1], bf16)
            nc.sync.dma_start(
                out=w1_sb[:].rearrange("p (k f) -> p k f", k=K0),
                in_=W1_d.rearrange("(k p) f -> p k f", p=P))
            w2_sb = const.tile([P, K2 * F2], bf16)
            nc.sync.dma_start(
                out=w2_sb[:].rearrange("p (k f) -> p k f", k=K2),
                in_=W2_d.rearrange("(k p) f -> p k f", p=P))
            b1_row = const.tile([1, F1], bf16)
            nc.sync.dma_start(out=b1_row[:, :], in_=b1_d[None, :])
            b2_row = const.tile([1, F2], bf16)
            nc.sync.dma_start(out=b2_row[:, :], in_=b2_d[None, :])

            iota_i = const.tile([P, P], i32)
            nc.gpsimd.iota(iota_i[:], pattern=[[1, P]], base=0,
                           channel_multiplier=0)
            iota_bf = const.tile([P, P], bf16)
            nc.vector.tensor_copy(out=iota_bf[:], in_=iota_i[:])
            ident = const.tile([P, P], bf16)
            make_identity(nc, ident[:])

            idxlo_sb = const.tile([P, lo_cols], i16)
            nc.sync.dma_start(out=idxlo_sb[:], in_=idxlo_d[:])
            idxhi_sb = const.tile([P, hi_cols], i16)
            nc.sync.dma_start(out=idxhi_sb[:], in_=idxhi_d[:])
            dstl_sb = const.tile([P, total], bf16)
            nc.sync.dma_start(out=dstl_sb[:], in_=dstl_d[:])
            dis_sb = const.tile([P, ntiles], f32)
            nc.sync.dma_start(out=dis_sb[:], in_=dis_d[:])
            invdis_sb = const.tile([1, RP], bf16)
            nc.sync.dma_start(out=invdis_sb[:], in_=invdis_d[:])

            h1T = const.tile([P, H1 * RP], bf16)

            # ---------------- phase 1: xw1' = dis * (x_c @ W1) --------------
            with (
                tc.tile_pool(name="p1x", bufs=1) as p1x,
                tc.tile_pool(name="p1o", bufs=3) as p1o,
                tc.tile_pool(name="p1ps", bufs=2, space="PSUM") as p1ps,
            ):
                xt_sb = p1x.tile([P, K0 * R], bf16)
                nc.sync.dma_start(
                    out=xt_sb[:].rearrange("p (k r) -> p k r", k=K0),
                    in_=xT.rearrange("(k p) r -> p k r", p=P))
                for m in range(ntiles):
                    rows = last_rows if m == ntiles - 1 else P
                    ps = p1ps.tile([P, F1], f32)
                    for k in range(K0):
                        nc.tensor.matmul(
                            out=ps[:rows, :],
                            lhsT=xt_sb[:, k * R + m * P: k * R + m * P + rows],
                            rhs=w1_sb[:, k * F1:(k + 1) * F1],
                            start=(k == 0), stop=(k == K0 - 1))
                    os = p1o.tile([P, F1], bf16)
                    nc.scalar.activation(out=os[:rows, :], in_=ps[:rows, :],
                                         func=mybir.ActivationFunctionType.Copy,
                                         scale=dis_sb[:rows, m:m + 1])
                    nc.sync.dma_start(out=ag1_in[m * P: m * P + rows, :],
                                      in_=os[:rows, :])

            nc.gpsimd.collective_compute(
                "AllGather", mybir.AluOpType.bypass, replica_groups=rg,
                ins=[ag1_in[:].opt()], outs=[ag1_out[:].opt()])

            # ------- phase 2: aggregate layer 1 (node-major), then
            #         relu+bias+dis and PE-transpose into h1T -------
            qctr = [0]

            def aggregate(t, ag_out, G_pool, S_pool, ps_pool, F):
                """Emit gathers + S build + matmul chain for tile t.
                Returns the psum tile (chain left open for the bias matmul)."""
                nch = chunks[t]
                j0 = offs[t]
                ps = ps_pool.tile([P, F], f32, tag="ps")
                G = G_pool.tile([P, maxch * F], bf16, tag="G")
                nlo, nhi = clo[t], chi[t]
                if nlo:
                    nc.gpsimd.dma_gather(
                        out_ap=G[:, :nlo * F].rearrange(
                            "p (c f) -> p c f", c=nlo),
                        in_ap=ag_out[:SPLIT, :],
                        idxs_ap=idxlo_sb[:, lo_offs[t] * 8:lo_offs[t + 1] * 8],
                        num_idxs=nlo * P, num_idxs_reg=nlo * P,
                        elem_size=F, single_packet=False,
                        queue_num=qctr[0] % 4)
                    qctr[0] += 1
                if nhi:
                    nc.gpsimd.dma_gather(
                        out_ap=G[:, nlo * F:nch * F].rearrange(
                            "p (c f) -> p c f", c=nhi),
                        in_ap=ag_out[SPLIT:, :],
                        idxs_ap=idxhi_sb[:, hi_offs[t] * 8:hi_offs[t + 1] * 8],
                        num_idxs=nhi * P, num_idxs_reg=nhi * P,
                        elem_size=F, single_packet=False,
                        queue_num=qctr[0] % 4)
                    qctr[0] += 1
                S = S_pool.tile([P, maxch * P], bf16, tag="S")
                nc.vector.tensor_tensor(
                    out=S[:, :nch * P].rearrange("p (c e) -> p c e", c=nch),
                    in0=iota_bf[:, None, :].to_broadcast([P, nch, P]),
                    in1=dstl_sb[:, j0:j0 + nch][:, :, None]
                        .to_broadcast([P, nch, P]),
                    op=mybir.AluOpType.is_equal)
                for c in range(nch):
                    nc.tensor.matmul(
                        out=ps[:], lhsT=S[:, c * P:(c + 1) * P],
                        rhs=G[:, c * F:(c + 1) * F],
                        start=(c == 0), stop=False)
                return ps, nch

            # phase 3 (hw2' = dis * (h1 @ W2)) is interleaved per tile right
            # after the tile's h1T is available, so it hides entirely under
            # the phase-2 gather stream and AG2 can fire the moment phase 2
            # ends.
            with (
                tc.tile_pool(name="p2g", bufs=5) as p2g,
                tc.tile_pool(name="p2s", bufs=3) as p2s,
                tc.tile_pool(name="p2h", bufs=3) as p2h,
                tc.tile_pool(name="p2ps", bufs=3, space="PSUM") as p2ps,
                tc.tile_pool(name="p2pt", bufs=3, space="PSUM") as p2pt,
                tc.tile_pool(name="p3o", bufs=3) as p3o,
                tc.tile_pool(name="p3ps", bufs=2, space="PSUM") as p3ps,
            ):
                for t in range(ntiles):
                    rows = last_rows if t == ntiles - 1 else P
                    ps, nch = aggregate(t, ag1_out, p2g, p2s, p2ps, F1)
                    nc.tensor.matmul(out=ps[:],
                                     lhsT=invdis_sb[:, t * P:(t + 1) * P],
                                     rhs=b1_row[:], start=(nch == 0),
                                     stop=True)
                    hm = p2h.tile([P, F1], bf16, tag="hm")
                    nc.scalar.activation(
                        out=hm[:], in_=ps[:],
                        func=mybir.ActivationFunctionType.Relu,
                        scale=dis_sb[:, t:t + 1])
                    for h in range(H1):
                        pt = p2pt.tile([P, P], bf16, tag="pt")
                        nc.tensor.transpose(
                            out=pt[:], in_=hm[:, h * P:(h + 1) * P],
                            identity=ident[:])
                        nc.vector.tensor_copy(
                            out=h1T[:, h * RP + t * P: h * RP + (t + 1) * P],
                            in_=pt[:])
                    # phase-3 matmul for this tile
                    ps3 = p3ps.tile([P, F2], f32, tag="ps3")
                    for k in range(K2):
                        nc.tensor.matmul(
                            out=ps3[:rows, :],
                            lhsT=h1T[:, k * RP + t * P: k * RP + t * P + rows],
                            rhs=w2_sb[:, k * F2:(k + 1) * F2],
                            start=(k == 0), stop=(k == K2 - 1))
                    os3 = p3o.tile([P, F2], bf16, tag="os3")
                    nc.scalar.activation(out=os3[:rows, :], in_=ps3[:rows, :],
                                         func=mybir.ActivationFunctionType.Copy,
                                         scale=dis_sb[:rows, t:t + 1])
                    nc.sync.dma_start(out=ag2_in[t * P: t * P + rows, :],
                                      in_=os3[:rows, :])

            nc.gpsimd.collective_compute(
                "AllGather", mybir.AluOpType.bypass, replica_groups=rg,
                ins=[ag2_in[:].opt()], outs=[ag2_out[:].opt()])

            # ------- phase 4: aggregate layer 2, node-major out -------
            with (
                tc.tile_pool(name="p4g", bufs=5) as p4g,
                tc.tile_pool(name="p4s", bufs=3) as p4s,
                tc.tile_pool(name="p4o", bufs=3) as p4o,
                tc.tile_pool(name="p4ps", bufs=3, space="PSUM") as p4ps,
            ):
                for t in range(ntiles):
                    rows = last_rows if t == ntiles - 1 else P
                    ps, nch = aggregate(t, ag2_out, p4g, p4s, p4ps, F2)
                    nc.tensor.matmul(out=ps[:],
                                     lhsT=invdis_sb[:, t * P:(t + 1) * P],
                                     rhs=b2_row[:], start=(nch == 0),
                                     stop=True)
                    os = p4o.tile([P, F2], bf16)
                    nc.scalar.activation(out=os[:rows, :], in_=ps[:rows, :],
                                         func=mybir.ActivationFunctionType.Copy,
                                         scale=dis_sb[:rows, t:t + 1])
                    nc.sync.dma_start(out=out_d[t * P: t * P + rows, :],
                                      in_=os[:rows, :])

    nc.compile()
    return nc


# ----------------------------------------------------------------------------
# Public entry point
# ----------------------------------------------------------------------------

LAST_EXEC_NS = None
LAST_RESULTS = None


def kernel(x, edge_index, W1, b1, W2, b2, _trace=False, _tmpdir=None):
    global LAST_EXEC_NS, LAST_RESULTS
    x = np.asarray(x, np.float32)
    edge_index = np.asarray(edge_index)
    W1 = np.asarray(W1, np.float32)
    b1 = np.asarray(b1, np.float32)
    W2 = np.asarray(W2, np.float32)
    b2 = np.asarray(b2, np.float32)
    N, F0 = x.shape
    F1 = W1.shape[1]
    F2 = W2.shape[1]

    packed, clo, chi, R, ntiles = _preprocess(x, edge_index, NCORES)
    nc = build_nc(N, R, ntiles, clo, chi, F0, F1, F2, NCORES)

    W1b = W1.astype(BF16)
    W2b = W2.astype(BF16)
    b1b = b1.astype(BF16)
    b2b = b2.astype(BF16)
    in_maps = []
    for c in range(NCORES):
        idx_lo, idx_hi, d_a, dis_t, invdis_r = packed[c]
        xT_c = np.ascontiguousarray(x[c * R:(c + 1) * R].T).astype(BF16)
        in_maps.append({
            "xT": xT_c, "idxlo": idx_lo, "idxhi": idx_hi, "dstl": d_a,
            "dis": dis_t, "invdis": invdis_r,
            "W1": W1b, "b1": b1b, "W2": W2b, "b2": b2b,
        })

    res = bass_utils.run_bass_kernel_spmd(
        nc, in_maps, core_ids=list(range(NCORES)), trace=_trace,
        tmpdir=_tmpdir)
    LAST_EXEC_NS = res.exec_time_ns
    LAST_RESULTS = res
    out = np.concatenate([res.results[c]["out"] for c in range(NCORES)], axis=0)
    return out.astype(np.float32)
